# Initial kernel scaffold
#
"""Trainium2 Bass kernel for nn_Conv3x3Block: conv3x3(64->128) + BN (folded) + LIF.

Strategy
--------
* Data-parallel over batch: B=16 -> 2 per core on 8 cores.
* Conv as 9 shifted matmuls accumulating in PSUM, contraction over Ci=64.
  Taps are paired along the partition axis (K=128) by stacking two copies of
  the zero-padded input image (second copy row-shifted by one padded row),
  so a single matmul computes two taps at once.
* BN is folded into the conv weights/bias host-side.
* LIF recurrence: membrane state is rescaled by a^-t, which folds the decay
  `a` into per-timestep weight/bias/threshold scalings.  The whole step
  m' = select(m<=th_t, m, 0) + y_t + bias_t  is ONE custom DVE instruction
  (reading y_t directly from PSUM).  Spikes are a stock is_gt tensor_scalar.
* Matmul precision modes:
    - "bf16x3": split x and w into bf16 hi+lo; compute x*wh (hi+lo stacked,
      K=128) + xh*wl correction. Error ~1e-6 relative: safe.
    - "f32r":  single-pass fp32r (tf32-class, ~1.5e-4 pre-threshold error).
    - "f32":   exact fp32 (4 cycles/row): slowest.
"""

import numpy as np
import ml_dtypes

import concourse.bass as bass
import concourse.mybir as mybir
import concourse.tile as tile
from concourse import bacc
from concourse.bass_utils import run_bass_kernel_spmd
from concourse.dve_spec import Spec, Src0, Src1, C0, C1, C2, Zero, select, lower
from concourse.dve_uop import DveOpSpec
import concourse.dve_ops as dve_ops

N_CORES = 8
T, B, CI, CO, H, W = 4, 16, 64, 128, 56, 56
HP, WP = H + 2, W + 2          # 58, 58
NPIX = H * W                   # 3136
NPAD = HP * WP                 # 3364
BLOC = B // N_CORES            # 2
BN_EPS = 1e-5

ROWS_PER_CHUNK = 8             # 8 out rows * 56 = 448 px per PSUM bank
CHUNK = ROWS_PER_CHUNK * W     # 448
HALF_A = [0, 1, 2, 3]          # chunk ids in half A (4 banks)
HALF_B = [4, 5, 6]             # chunk ids in half B (3 banks)

MODE = "f32r"                  # primary mode (rel err ~1.1e-2 < 2e-2 gate)
SBUF_COPY = True               # build 2nd stacked x half by SBUF->SBUF DMA
SPK_BF16 = True                # spike DMA in bf16 (0/1 exact), cast on host
S_ON_ACT = False               # spike op on ScalarE (relu(sign(u-th))) vs DVE

f32 = mybir.dt.float32
f32r = mybir.dt.float32r
bf16 = mybir.dt.bfloat16
BF16 = ml_dtypes.bfloat16


# --------------------------------------------------------------------------- #
# custom DVE op: one fused LIF step (decay pre-folded via state rescaling)
# --------------------------------------------------------------------------- #

def _register_op(name, spec_body, reference):
    if name in dve_ops._SUB_OPCODE_FOR_NAME:
        return next(o for o in dve_ops.OPS if o.name == name)
    spec = Spec(body=spec_body, reference=reference)
    row = dve_ops._CUSTOM_DVE_ROW_BASE + len(dve_ops.OPS)
    assert row < 0x20
    shas = {}
    for ver in ("v3", "v4"):
        tmp = DveOpSpec(name=name, opcode=row, uops=lower(spec, ver=ver),
                        rd1_en=True)
        shas[ver] = tmp.sha(ver)
    op = dve_ops.DveOp(name, spec, subdim=False, uops_sha=shas)
    dve_ops.OPS.append(op)
    dve_ops.CUSTOM_DVE_SPECS[name] = spec
    dve_ops._SUB_OPCODE_FOR_NAME[name] = row
    return op


def lif_op():
    # m' = (m <= th ? m : 0) + y + bias      (in0=m, in1=y, s0=bias, s1=th)
    return _register_op(
        "LIF_STEP_Z_ANT",
        select(Src0 <= C1, Src0, Zero) + Src1 + C0,
        lambda in0, in1, s0, s1, imm2: (
            np.where(in0.astype(np.float32) <= s1, in0, 0.0).astype(np.float32)
            + in1.reshape(in0.shape[0], -1).astype(np.float32) + s0
        ).astype(np.float32),
    )


# --------------------------------------------------------------------------- #
# per-core SPMD module
# --------------------------------------------------------------------------- #

def _mm_groups(mode):
    """(wname, widx, dy, dx, k, xtile_kind) per matmul group."""
    gs = []
    if mode in ("f32r", "f32"):
        for dx in range(3):
            gs.append(("wp", dx, 0, dx, 128, "X2"))   # taps (0,dx)+(1,dx)
        for dx in range(3):
            gs.append(("ws", dx, 2, dx, 64, "X2"))    # taps (2,dx)
    else:  # bf16x3
        for dy in range(3):
            for dx in range(3):
                gs.append(("w3", 3 * dy + dx, dy, dx, 128, "T3"))  # x*wh
        for dx in range(3):
            gs.append(("wc", dx, 0, dx, 128, "T1"))   # xh*wl taps (0,dx)+(1,dx)
        for dx in range(3):
            gs.append(("wcs", dx, 2, dx, 64, "T3"))   # xh*wl taps (2,dx)
    return gs


def build_module(mode, th_ts, repeat=1, hw_loop=0):
    op = lif_op()
    mdt = {"f32r": f32r, "f32": f32, "bf16x3": bf16}[mode]
    groups = _mm_groups(mode)

    nc = bacc.Bacc(None, target_bir_lowering=False)
    with tile.TileContext(nc) as tc:
        with tc.tile_pool(name="dram", bufs=1, space="DRAM") as dram:
            # ---- DRAM I/O  (weights already in SBUF layout [K, T, G, CO]) ---
            xins = {}
            if mode in ("f32r", "f32"):
                xins["xp"] = dram.tile([T, BLOC, CI, HP, WP], mdt,
                                       kind="ExternalInput", name="xp", uniquify=False)
            else:
                xins["xh"] = dram.tile([T, BLOC, CI, HP, WP], bf16,
                                       kind="ExternalInput", name="xh", uniquify=False)
                xins["xl"] = dram.tile([T, BLOC, CI, HP, WP], bf16,
                                       kind="ExternalInput", name="xl", uniquify=False)
            wins = {}
            if mode in ("f32r", "f32"):
                wins["wp"] = dram.tile([128, T, 3, CO], mdt,
                                       kind="ExternalInput", name="wp", uniquify=False)
                wins["ws"] = dram.tile([64, T, 3, CO], mdt,
                                       kind="ExternalInput", name="ws", uniquify=False)
            else:
                wins["w3"] = dram.tile([128, T, 9, CO], bf16,
                                       kind="ExternalInput", name="w3", uniquify=False)
                wins["wc"] = dram.tile([128, T, 3, CO], bf16,
                                       kind="ExternalInput", name="wc", uniquify=False)
                wins["wcs"] = dram.tile([64, T, 3, CO], bf16,
                                        kind="ExternalInput", name="wcs", uniquify=False)
            bias_in = dram.tile([CO, T], f32, kind="ExternalInput",
                                name="bias_in", uniquify=False)
            sdt = bf16 if SPK_BF16 else f32
            spk = dram.tile([T, BLOC, CO, NPIX], sdt, kind="ExternalOutput",
                            name="spk", uniquify=False)

            with (
                tc.tile_pool(name="wpool", bufs=1) as wpool,
                tc.tile_pool(name="const", bufs=1) as cpool,
                tc.tile_pool(name="xpool", bufs=8) as xpool,
                tc.tile_pool(name="upool", bufs=3) as upool,
                tc.tile_pool(name="spool", bufs=3) as spool,
                tc.tile_pool(name="ps", bufs=1, space="PSUM") as pspool,
            ):
                # ---- persistent weights / bias / zero tile ------------------
                wsb = {}
                for wn, win in wins.items():
                    t_ = wpool.tile(list(win.shape), win.dtype, name=f"{wn}_sb")
                    nc.sync.dma_start(t_[:], win[:])
                    wsb[wn] = t_
                bias_sb = cpool.tile([CO, T], f32, name="bias_sb")
                nc.sync.dma_start(bias_sb[:], bias_in[:])
                zt = cpool.tile([128, 4, CHUNK], f32, name="zt")
                nc.vector.memset(zt[:], 0.0)

                # ---- main loops --------------------------------------------
                import contextlib
                loop_cm = tc.For_i(0, hw_loop, 1) if hw_loop else contextlib.nullcontext()
                with loop_cm:
                  for rep in range(repeat):
                    for b in range(BLOC):
                        xts = {}
                        for t in range(T):
                            if mode in ("f32r", "f32"):
                                x2 = xpool.tile([128, HP, WP], mdt,
                                                name=f"x2_{rep}_{b}_{t}", tag="x2")
                                nc.sync.dma_start(x2[0:CI], xins["xp"][t, b])
                                if SBUF_COPY:
                                    nc.sync.dma_start(
                                        x2.rearrange("p h w -> p (h w)")[CI:128, 0:NPAD - WP],
                                        x2.rearrange("p h w -> p (h w)")[0:CI, WP:NPAD])
                                else:
                                    nc.sync.dma_start(
                                        x2.rearrange("p h w -> p (h w)")[CI:128, 0:NPAD - WP],
                                        xins["xp"][t, b].rearrange("c h w -> c (h w)")[:, WP:NPAD])
                                xts[("X2", t)] = x2
                            else:
                                t3 = xpool.tile([128, HP, WP], bf16,
                                                name=f"t3_{rep}_{b}_{t}", tag="t3")
                                nc.sync.dma_start(t3[0:CI], xins["xh"][t, b])
                                nc.sync.dma_start(t3[CI:128], xins["xl"][t, b])
                                t1 = xpool.tile([128, HP, WP], bf16,
                                                name=f"t1_{rep}_{b}_{t}", tag="t1")
                                nc.sync.dma_start(t1[0:CI], xins["xh"][t, b])
                                if SBUF_COPY:
                                    nc.sync.dma_start(
                                        t1.rearrange("p h w -> p (h w)")[CI:128, 0:NPAD - WP],
                                        t1.rearrange("p h w -> p (h w)")[0:CI, WP:NPAD])
                                else:
                                    nc.sync.dma_start(
                                        t1.rearrange("p h w -> p (h w)")[CI:128, 0:NPAD - WP],
                                        xins["xh"][t, b].rearrange("c h w -> c (h w)")[:, WP:NPAD])
                                xts[("T3", t)] = t3
                                xts[("T1", t)] = t1

                        u_prev = None
                        for t in range(T):
                            u_t = upool.tile([128, NPIX], f32,
                                             name=f"u_{rep}_{b}_{t}", tag="u")
                            for half, chunks in (("A", HALF_A), ("B", HALF_B)):
                                nch = len(chunks)
                                ps = pspool.tile([128, nch, 512], f32,
                                                 name=f"ps{half}_{rep}_{b}_{t}",
                                                 tag=f"ps{half}")
                                ng = len(groups)
                                for gi, (wn, widx, dy, dx, k, xkind) in enumerate(groups):
                                    lhsT = wsb[wn][0:k, t, widx, :]
                                    xt = xts[(xkind, t)]
                                    for ci, c in enumerate(chunks):
                                        r0 = c * ROWS_PER_CHUNK
                                        rhs = xt[0:k, r0 + dy: r0 + dy + ROWS_PER_CHUNK,
                                                 dx: dx + W]
                                        nc.tensor.matmul(
                                            ps[:, ci, 0:CHUNK], lhsT, rhs,
                                            start=(gi == 0), stop=(gi == ng - 1))
                                # fused LIF step for this half
                                lo = chunks[0] * CHUNK
                                hi = lo + nch * CHUNK
                                in0 = (zt[:, 0:nch, :] if t == 0
                                       else u_prev[:, lo:hi].rearrange(
                                           "p (s n) -> p s n", n=CHUNK))
                                nc.vector._custom_dve(
                                    op,
                                    out=u_t[:, lo:hi].rearrange("p (s n) -> p s n", n=CHUNK),
                                    in0=in0, in1=ps[:, :, 0:CHUNK],
                                    s0=bias_sb[:, t:t + 1],
                                    s1=float(th_ts[t - 1] if t > 0 else th_ts[0]))
                            # spikes
                            s_t = spool.tile([128, NPIX], sdt,
                                             name=f"s_{rep}_{b}_{t}", tag="s")
                            if S_ON_ACT:
                                sg = spool.tile([128, NPIX], f32,
                                                name=f"sg_{rep}_{b}_{t}", tag="sg")
                                nc.scalar.activation(
                                    sg[:], u_t[:],
                                    mybir.ActivationFunctionType.Sign,
                                    bias=-float(th_ts[t]), scale=1.0)
                                nc.scalar.activation(
                                    s_t[:], sg[:],
                                    mybir.ActivationFunctionType.Relu)
                            else:
                                nc.vector.tensor_scalar(
                                    s_t[:], u_t[:], float(th_ts[t]), None,
                                    mybir.AluOpType.is_gt)
                            nc.sync.dma_start(spk[t, b], s_t[:])
                            u_prev = u_t
    nc.compile()
    return nc


# --------------------------------------------------------------------------- #
# host-side input prep
# --------------------------------------------------------------------------- #

def _prep(mode, x, conv_w, conv_b, bn_gamma, bn_beta, bn_mean, bn_var,
          alpha, lif_beta, theta, v0):
    a = float(np.asarray(alpha).reshape(-1)[0])
    bb = float(np.asarray(lif_beta).reshape(-1)[0])
    th = float(np.asarray(theta).reshape(-1)[0])
    vr = float(np.asarray(v0).reshape(-1)[0])
    assert vr == 0.0, "fast path assumes v0 == 0"
    assert a > 0.0, "fast path assumes alpha > 0"

    scale = (bn_gamma / np.sqrt(bn_var + BN_EPS)).astype(np.float32)
    wf = (conv_w * scale[:, None, None, None]).astype(np.float32)   # [CO,CI,3,3]
    biasf = ((conv_b - bn_mean) * scale + bn_beta).astype(np.float32)

    fac = np.array([a ** (-t) for t in range(T)], dtype=np.float64)
    th_ts = tuple(float(np.float32(th * fac[t])) for t in range(T))

    # bias per t (scaled); t=0 additionally gets a*vr (zero here)
    bias_np = np.empty((CO, T), np.float32)
    for t in range(T):
        bias_np[:, t] = ((biasf + bb) * fac[t]).astype(np.float32)
    bias_np[:, 0] += np.float32(a * vr)

    # padded input
    xpad = np.zeros((T, B, CI, HP, WP), np.float32)
    xpad[:, :, :, 1:H + 1, 1:W + 1] = np.asarray(x, dtype=np.float32)

    ins = {}
    if mode in ("f32r", "f32"):
        ins["xp"] = xpad
        wp = np.empty((128, T, 3, CO), np.float32)
        ws = np.empty((64, T, 3, CO), np.float32)
        for t in range(T):
            wt = (wf * np.float32(fac[t])).astype(np.float32)
            for dx in range(3):
                wp[0:64, t, dx, :] = wt[:, :, 0, dx].T
                wp[64:128, t, dx, :] = wt[:, :, 1, dx].T
                ws[:, t, dx, :] = wt[:, :, 2, dx].T
        ins["wp"], ins["ws"] = wp, ws
    else:
        xh = xpad.astype(BF16)
        xl = (xpad - xh.astype(np.float32)).astype(BF16)
        ins["xh"], ins["xl"] = xh, xl
        w3 = np.empty((128, T, 9, CO), BF16)
        wc = np.empty((128, T, 3, CO), BF16)
        wcs = np.empty((64, T, 3, CO), BF16)
        for t in range(T):
            wt = (wf * np.float32(fac[t])).astype(np.float32)
            wh = wt.astype(BF16)
            wl = (wt - wh.astype(np.float32)).astype(BF16)
            for dy in range(3):
                for dx in range(3):
                    w3[0:64, t, 3 * dy + dx, :] = wh[:, :, dy, dx].T
                    w3[64:128, t, 3 * dy + dx, :] = wh[:, :, dy, dx].T
            for dx in range(3):
                wc[0:64, t, dx, :] = wl[:, :, 0, dx].T
                wc[64:128, t, dx, :] = wl[:, :, 1, dx].T
                wcs[:, t, dx, :] = wl[:, :, 2, dx].T
        ins["w3"], ins["wc"], ins["wcs"] = w3, wc, wcs

    ins["bias_in"] = bias_np
    return ins, th_ts


def _in_maps(mode, ins):
    """Split full prepped inputs into 8 per-core maps (shard along B)."""
    maps = []
    for c in range(N_CORES):
        m = {}
        for k, v in ins.items():
            if k in ("xp", "xh", "xl"):
                m[k] = np.ascontiguousarray(v[:, c * BLOC:(c + 1) * BLOC])
            else:
                m[k] = v
        maps.append(m)
    return maps


# --------------------------------------------------------------------------- #
# public entry
# --------------------------------------------------------------------------- #

_CACHE = {}


def _get_module(mode, th_ts, repeat=1, hw_loop=0):
    key = (mode, th_ts, repeat, hw_loop, SBUF_COPY, SPK_BF16, S_ON_ACT)
    if key not in _CACHE:
        _CACHE[key] = build_module(mode, th_ts, repeat, hw_loop)
    return _CACHE[key]


def _run(mode, inputs, repeat=1):
    ins, th_ts = _prep(mode, **inputs)
    nc = _get_module(mode, th_ts, repeat)
    maps = _in_maps(mode, ins)
    res = run_bass_kernel_spmd(nc, maps, core_ids=list(range(N_CORES)))
    out = np.concatenate([r["spk"] for r in res.results], axis=1)
    return out.reshape(T, B, CO, H, W).astype(np.float32)


def kernel(**inputs):
    inputs = {k: np.asarray(v) for k, v in inputs.items()}
    return _run(MODE, inputs)



# revision 9
# speedup vs baseline: 1.5844x; 1.5844x over previous
"""Trainium2 Bass kernel for nn_Conv3x3Block: conv3x3(64->128) + BN (folded) + LIF.

Strategy
--------
* Data-parallel over batch: B=16 -> 2 per core on 8 cores.
* Conv as 9 shifted matmuls accumulating in PSUM, contraction over Ci=64.
  Taps are paired along the partition axis (K=128) by stacking two copies of
  the zero-padded input image (second copy row-shifted by one padded row),
  so a single matmul computes two taps at once.
* BN is folded into the conv weights/bias host-side.
* LIF recurrence: membrane state is rescaled by a^-t, which folds the decay
  `a` into per-timestep weight/bias/threshold scalings.  The whole step
  m' = select(m<=th_t, m, 0) + y_t + bias_t  is ONE custom DVE instruction
  (reading y_t directly from PSUM).  Spikes are a stock is_gt tensor_scalar.
* Matmul precision modes:
    - "bf16x3": split x and w into bf16 hi+lo; compute x*wh (hi+lo stacked,
      K=128) + xh*wl correction. Error ~1e-6 relative: safe.
    - "f32r":  single-pass fp32r (tf32-class, ~1.5e-4 pre-threshold error).
    - "f32":   exact fp32 (4 cycles/row): slowest.
"""

import numpy as np
import ml_dtypes

import concourse.bass as bass
import concourse.mybir as mybir
import concourse.tile as tile
from concourse import bacc
from concourse.bass_utils import run_bass_kernel_spmd
from concourse.dve_spec import Spec, Src0, Src1, C0, C1, C2, Zero, select, lower
from concourse.dve_uop import DveOpSpec
import concourse.dve_ops as dve_ops

N_CORES = 8
T, B, CI, CO, H, W = 4, 16, 64, 128, 56, 56
HP, WP = H + 2, W + 2          # 58, 58
NPIX = H * W                   # 3136
NPAD = HP * WP                 # 3364
BLOC = B // N_CORES            # 2
BN_EPS = 1e-5

ROWS_PER_CHUNK = 8             # 8 out rows * 56 = 448 px per PSUM bank
CHUNK = ROWS_PER_CHUNK * W     # 448
HALF_A = [0, 1, 2, 3]          # chunk ids in half A (4 banks)
HALF_B = [4, 5, 6]             # chunk ids in half B (3 banks)

MODE = "f32r"                  # primary mode (rel err ~1.1e-2 < 2e-2 gate)
SBUF_COPY = True               # build 2nd stacked x half by SBUF->SBUF DMA
SPK_BF16 = True                # spike DMA in bf16 (0/1 exact), cast on host
S_ON_ACT = False               # spike op on ScalarE (relu(sign(u-th))) vs DVE

f32 = mybir.dt.float32
f32r = mybir.dt.float32r
bf16 = mybir.dt.bfloat16
BF16 = ml_dtypes.bfloat16


# --------------------------------------------------------------------------- #
# custom DVE op: one fused LIF step (decay pre-folded via state rescaling)
# --------------------------------------------------------------------------- #

def _register_op(name, spec_body, reference):
    if name in dve_ops._SUB_OPCODE_FOR_NAME:
        return next(o for o in dve_ops.OPS if o.name == name)
    spec = Spec(body=spec_body, reference=reference)
    row = dve_ops._CUSTOM_DVE_ROW_BASE + len(dve_ops.OPS)
    assert row < 0x20
    shas = {}
    for ver in ("v3", "v4"):
        tmp = DveOpSpec(name=name, opcode=row, uops=lower(spec, ver=ver),
                        rd1_en=True)
        shas[ver] = tmp.sha(ver)
    op = dve_ops.DveOp(name, spec, subdim=False, uops_sha=shas)
    dve_ops.OPS.append(op)
    dve_ops.CUSTOM_DVE_SPECS[name] = spec
    dve_ops._SUB_OPCODE_FOR_NAME[name] = row
    return op


def lif_op():
    # m' = (m <= th ? m : 0) + y + bias      (in0=m, in1=y, s0=bias, s1=th)
    return _register_op(
        "LIF_STEP_Z_ANT",
        select(Src0 <= C1, Src0, Zero) + Src1 + C0,
        lambda in0, in1, s0, s1, imm2: (
            np.where(in0.astype(np.float32) <= s1, in0, 0.0).astype(np.float32)
            + in1.reshape(in0.shape[0], -1).astype(np.float32) + s0
        ).astype(np.float32),
    )


# --------------------------------------------------------------------------- #
# per-core SPMD module
# --------------------------------------------------------------------------- #

def _mm_groups(mode):
    """(wname, widx, dy, dx, k, xtile_kind) per matmul group."""
    gs = []
    if mode in ("f32r", "f32"):
        for dx in range(3):
            gs.append(("wp", dx, 0, dx, 128, "X2"))   # taps (0,dx)+(1,dx)
        for dx in range(3):
            gs.append(("ws", dx, 2, dx, 64, "X2"))    # taps (2,dx)
    else:  # bf16x3
        for dy in range(3):
            for dx in range(3):
                gs.append(("w3", 3 * dy + dx, dy, dx, 128, "T3"))  # x*wh
        for dx in range(3):
            gs.append(("wc", dx, 0, dx, 128, "T1"))   # xh*wl taps (0,dx)+(1,dx)
        for dx in range(3):
            gs.append(("wcs", dx, 2, dx, 64, "T3"))   # xh*wl taps (2,dx)
    return gs


def build_module(mode, th_ts, repeat=1, hw_loop=0):
    op = lif_op()
    mdt = {"f32r": f32r, "f32": f32, "bf16x3": bf16}[mode]
    groups = _mm_groups(mode)

    nc = bacc.Bacc(None, target_bir_lowering=False)
    with tile.TileContext(nc) as tc:
        with tc.tile_pool(name="dram", bufs=1, space="DRAM") as dram:
            # ---- DRAM I/O  (weights already in SBUF layout [K, T, G, CO]) ---
            xins = {}
            if mode in ("f32r", "f32"):
                xins["xp"] = dram.tile([T, BLOC, CI, HP, WP], mdt,
                                       kind="ExternalInput", name="xp", uniquify=False)
            else:
                xins["xh"] = dram.tile([T, BLOC, CI, HP, WP], bf16,
                                       kind="ExternalInput", name="xh", uniquify=False)
                xins["xl"] = dram.tile([T, BLOC, CI, HP, WP], bf16,
                                       kind="ExternalInput", name="xl", uniquify=False)
            wins = {}
            if mode in ("f32r", "f32"):
                wins["wp"] = dram.tile([128, T, 3, CO], mdt,
                                       kind="ExternalInput", name="wp", uniquify=False)
                wins["ws"] = dram.tile([64, T, 3, CO], mdt,
                                       kind="ExternalInput", name="ws", uniquify=False)
            else:
                wins["w3"] = dram.tile([128, T, 9, CO], bf16,
                                       kind="ExternalInput", name="w3", uniquify=False)
                wins["wc"] = dram.tile([128, T, 3, CO], bf16,
                                       kind="ExternalInput", name="wc", uniquify=False)
                wins["wcs"] = dram.tile([64, T, 3, CO], bf16,
                                        kind="ExternalInput", name="wcs", uniquify=False)
            bias_in = dram.tile([CO, T], f32, kind="ExternalInput",
                                name="bias_in", uniquify=False)
            sdt = bf16 if SPK_BF16 else f32
            spk = dram.tile([T, BLOC, CO, NPIX], sdt, kind="ExternalOutput",
                            name="spk", uniquify=False)

            with (
                tc.tile_pool(name="wpool", bufs=1) as wpool,
                tc.tile_pool(name="const", bufs=1) as cpool,
                tc.tile_pool(name="xpool", bufs=8) as xpool,
                tc.tile_pool(name="upool", bufs=3) as upool,
                tc.tile_pool(name="spool", bufs=3) as spool,
                tc.tile_pool(name="ps", bufs=1, space="PSUM") as pspool,
            ):
                # ---- persistent weights / bias / zero tile ------------------
                wsb = {}
                for wn, win in wins.items():
                    t_ = wpool.tile(list(win.shape), win.dtype, name=f"{wn}_sb")
                    nc.sync.dma_start(t_[:], win[:])
                    wsb[wn] = t_
                bias_sb = cpool.tile([CO, T], f32, name="bias_sb")
                nc.sync.dma_start(bias_sb[:], bias_in[:])
                zt = cpool.tile([128, 4, CHUNK], f32, name="zt")
                nc.vector.memset(zt[:], 0.0)

                # ---- main loops --------------------------------------------
                import contextlib
                loop_cm = tc.For_i(0, hw_loop, 1) if hw_loop else contextlib.nullcontext()
                with loop_cm:
                  for rep in range(repeat):
                    for b in range(BLOC):
                        xts = {}
                        for t in range(T):
                            if mode in ("f32r", "f32"):
                                x2 = xpool.tile([128, HP, WP], mdt,
                                                name=f"x2_{rep}_{b}_{t}", tag="x2")
                                nc.sync.dma_start(x2[0:CI], xins["xp"][t, b])
                                if SBUF_COPY:
                                    nc.sync.dma_start(
                                        x2.rearrange("p h w -> p (h w)")[CI:128, 0:NPAD - WP],
                                        x2.rearrange("p h w -> p (h w)")[0:CI, WP:NPAD])
                                else:
                                    nc.sync.dma_start(
                                        x2.rearrange("p h w -> p (h w)")[CI:128, 0:NPAD - WP],
                                        xins["xp"][t, b].rearrange("c h w -> c (h w)")[:, WP:NPAD])
                                xts[("X2", t)] = x2
                            else:
                                t3 = xpool.tile([128, HP, WP], bf16,
                                                name=f"t3_{rep}_{b}_{t}", tag="t3")
                                nc.sync.dma_start(t3[0:CI], xins["xh"][t, b])
                                nc.sync.dma_start(t3[CI:128], xins["xl"][t, b])
                                t1 = xpool.tile([128, HP, WP], bf16,
                                                name=f"t1_{rep}_{b}_{t}", tag="t1")
                                nc.sync.dma_start(t1[0:CI], xins["xh"][t, b])
                                if SBUF_COPY:
                                    nc.sync.dma_start(
                                        t1.rearrange("p h w -> p (h w)")[CI:128, 0:NPAD - WP],
                                        t1.rearrange("p h w -> p (h w)")[0:CI, WP:NPAD])
                                else:
                                    nc.sync.dma_start(
                                        t1.rearrange("p h w -> p (h w)")[CI:128, 0:NPAD - WP],
                                        xins["xh"][t, b].rearrange("c h w -> c (h w)")[:, WP:NPAD])
                                xts[("T3", t)] = t3
                                xts[("T1", t)] = t1

                        u_prev = None
                        for t in range(T):
                            u_t = upool.tile([128, NPIX], f32,
                                             name=f"u_{rep}_{b}_{t}", tag="u")
                            for half, chunks in (("A", HALF_A), ("B", HALF_B)):
                                nch = len(chunks)
                                ps = pspool.tile([128, nch, 512], f32,
                                                 name=f"ps{half}_{rep}_{b}_{t}",
                                                 tag=f"ps{half}")
                                ng = len(groups)
                                for gi, (wn, widx, dy, dx, k, xkind) in enumerate(groups):
                                    lhsT = wsb[wn][0:k, t, widx, :]
                                    xt = xts[(xkind, t)]
                                    for ci, c in enumerate(chunks):
                                        r0 = c * ROWS_PER_CHUNK
                                        rhs = xt[0:k, r0 + dy: r0 + dy + ROWS_PER_CHUNK,
                                                 dx: dx + W]
                                        nc.tensor.matmul(
                                            ps[:, ci, 0:CHUNK], lhsT, rhs,
                                            start=(gi == 0), stop=(gi == ng - 1))
                                # fused LIF step for this half
                                lo = chunks[0] * CHUNK
                                hi = lo + nch * CHUNK
                                in0 = (zt[:, 0:nch, :] if t == 0
                                       else u_prev[:, lo:hi].rearrange(
                                           "p (s n) -> p s n", n=CHUNK))
                                nc.vector._custom_dve(
                                    op,
                                    out=u_t[:, lo:hi].rearrange("p (s n) -> p s n", n=CHUNK),
                                    in0=in0, in1=ps[:, :, 0:CHUNK],
                                    s0=bias_sb[:, t:t + 1],
                                    s1=float(th_ts[t - 1] if t > 0 else th_ts[0]))
                            # spikes
                            s_t = spool.tile([128, NPIX], sdt,
                                             name=f"s_{rep}_{b}_{t}", tag="s")
                            if S_ON_ACT:
                                sg = spool.tile([128, NPIX], f32,
                                                name=f"sg_{rep}_{b}_{t}", tag="sg")
                                nc.scalar.activation(
                                    sg[:], u_t[:],
                                    mybir.ActivationFunctionType.Sign,
                                    bias=-float(th_ts[t]), scale=1.0)
                                nc.scalar.activation(
                                    s_t[:], sg[:],
                                    mybir.ActivationFunctionType.Relu)
                            else:
                                nc.vector.tensor_scalar(
                                    s_t[:], u_t[:], float(th_ts[t]), None,
                                    mybir.AluOpType.is_gt)
                            nc.sync.dma_start(spk[t, b], s_t[:])
                            u_prev = u_t
    nc.compile()
    return nc


# --------------------------------------------------------------------------- #
# v2 "bp" mode: batch-pair row-tiled K=64 matmuls
# --------------------------------------------------------------------------- #
#  * b0's x on SBUF partitions 0-63, b1's on 64-127; 9 plain K=64 matmuls per
#    chunk per image, issued b0/b1-alternated so the two PE row-groups (0-63 /
#    64-127) stream concurrently (tile_position auto-derived from base
#    partitions).  No stacked-copy DMA at all.
#  * PSUM: [128, 2, 512] tiles per (chunk-pair, image): 2+2+2+1 chunks,
#    2 pairs in flight = 8 banks.
#  * LIF custom DVE op at chunk-pair granularity reading PSUM directly.
#  * Spikes: ScalarE Sign(u - th) -> {-1,0,+1}, fp8e4 output, host maps >0.

CPAIRS = [(0, 1), (2, 3), (4, 5), (6,)]
BP_F16 = True                   # conv operands in fp16 (10-bit mantissa, like
                                # tf32) instead of f32r: halves x DMA traffic
f16 = mybir.dt.float16


def build_module_bp(th_ts, repeat=1, hw_loop=0, sdt=None):
    op = lif_op()
    mdt = f16 if BP_F16 else f32r
    if sdt is None:
        sdt = mybir.dt.float8e4
    nc = bacc.Bacc(None, target_bir_lowering=False)
    with tile.TileContext(nc) as tc:
        with tc.tile_pool(name="dram", bufs=1, space="DRAM") as dram:
            xin = dram.tile([T, BLOC, CI, HP, WP], mdt, kind="ExternalInput",
                            name="xp", uniquify=False)
            win = dram.tile([128, T, 9, CO], mdt, kind="ExternalInput",
                            name="wb", uniquify=False)
            bias_in = dram.tile([CO, T], f32, kind="ExternalInput",
                                name="bias_in", uniquify=False)
            nth_in = dram.tile([CO, T], f32, kind="ExternalInput",
                               name="nth_in", uniquify=False)
            spk = dram.tile([T, BLOC, CO, NPIX], sdt, kind="ExternalOutput",
                            name="spk", uniquify=False)

            with (
                tc.tile_pool(name="wpool", bufs=1) as wpool,
                tc.tile_pool(name="const", bufs=1) as cpool,
                tc.tile_pool(name="xpool", bufs=2) as xpool,
                tc.tile_pool(name="upool", bufs=2) as upool,
                tc.tile_pool(name="spool", bufs=2) as spool,
                tc.tile_pool(name="ps", bufs=1, space="PSUM") as pspool,
            ):
                wsb = wpool.tile([128, T, 9, CO], mdt, name="wb_sb")
                bias_sb = cpool.tile([CO, T], f32, name="bias_sb")
                nth_sb = cpool.tile([CO, T], f32, name="nth_sb")
                zt = cpool.tile([128, 2, CHUNK], f32, name="zt")

                # row pieces sized so chunk-pair p only waits on pieces <= p
                XPIECES = [(0, 18), (18, 34), (34, 50), (50, 58)]

                def load_x(rep, t, split=True):
                    xt = xpool.tile([128, HP, WP], mdt,
                                    name=f"x_{rep}_{t}", tag="x")
                    for b in range(BLOC):
                        if split:
                            for r0, r1 in XPIECES:
                                nc.sync.dma_start(
                                    xt[b * CI:(b + 1) * CI, r0:r1],
                                    xin[t, b][:, r0:r1])
                        else:
                            nc.sync.dma_start(xt[b * CI:(b + 1) * CI], xin[t, b])
                    return xt

                import contextlib
                loop_cm = tc.For_i(0, hw_loop, 1) if hw_loop else contextlib.nullcontext()
                first = True
                with loop_cm:
                  for rep in range(repeat):
                    if first:
                        # head: x for t=0 first, then t=0 weights, bias, zeros
                        xcur = load_x(rep, 0)
                        nc.sync.dma_start(wsb[:, 0], win[:, 0])
                        nc.sync.dma_start(bias_sb[:], bias_in[:])
                        nc.sync.dma_start(nth_sb[:], nth_in[:])
                        nc.vector.memset(zt[:], 0.0)
                        for t in range(1, T):
                            nc.sync.dma_start(wsb[:, t], win[:, t])
                        first = False
                    elif xcur is None:
                        xcur = load_x(rep, 0)
                    u_prev = None
                    for t in range(T):
                        # prefetch next x (monolithic: overlaps compute anyway)
                        if t + 1 < T:
                            xnext = load_x(rep, t + 1, split=False)
                        elif rep + 1 < repeat:
                            xnext = load_x(rep + 1, 0, split=False)
                        else:
                            xnext = None
                        u_cur = {b: upool.tile([128, NPIX], f32,
                                               name=f"u_{rep}_{t}_{b}", tag=f"u{b}")
                                 for b in range(BLOC)}
                        s_t = {b: spool.tile([128, NPIX], sdt,
                                             name=f"s_{rep}_{t}_{b}", tag=f"s{b}")
                               for b in range(BLOC)}
                        last_t = t == T - 1
                        for p, cp in enumerate(CPAIRS):
                            nch = len(cp)
                            ps = {b: pspool.tile([128, nch, 512], f32,
                                                 name=f"ps_{rep}_{t}_{p}_{b}",
                                                 tag=f"ps{p % 2}_{b}")
                                  for b in range(BLOC)}
                            for s9 in range(9):
                                dy, dx = divmod(s9, 3)
                                for ci, c in enumerate(cp):
                                    r0 = c * ROWS_PER_CHUNK
                                    for b in range(BLOC):
                                        p0 = b * CI
                                        nc.tensor.matmul(
                                            ps[b][:, ci, 0:CHUNK],
                                            wsb[p0:p0 + CI, t, s9, :],
                                            xcur[p0:p0 + CI,
                                                 r0 + dy: r0 + dy + ROWS_PER_CHUNK,
                                                 dx: dx + W],
                                            start=(s9 == 0), stop=(s9 == 8))
                            lo = cp[0] * CHUNK
                            hi = lo + nch * CHUNK
                            for b in range(BLOC):
                                in0 = (zt[:, 0:nch, :] if t == 0
                                       else u_prev[b][:, lo:hi].rearrange(
                                           "p (s n) -> p s n", n=CHUNK))
                                nc.vector._custom_dve(
                                    op,
                                    out=u_cur[b][:, lo:hi].rearrange(
                                        "p (s n) -> p s n", n=CHUNK),
                                    in0=in0, in1=ps[b][:, :, 0:CHUNK],
                                    s0=bias_sb[:, t:t + 1],
                                    s1=float(th_ts[t - 1] if t > 0 else th_ts[0]))
                                if last_t:
                                    # fine-grained spikes+DMA to shrink tail
                                    nc.scalar.activation(
                                        s_t[b][:, lo:hi], u_cur[b][:, lo:hi],
                                        mybir.ActivationFunctionType.Sign,
                                        bias=nth_sb[:, t:t + 1], scale=1.0)
                                    nc.sync.dma_start(
                                        spk[t, b].rearrange(
                                            "c (s n) -> c s n", n=CHUNK
                                        )[:, cp[0]:cp[0] + nch, :],
                                        s_t[b][:, lo:hi].rearrange(
                                            "p (s n) -> p s n", n=CHUNK))
                        if not last_t:
                            for b in range(BLOC):
                                nc.scalar.activation(
                                    s_t[b][:], u_cur[b][:],
                                    mybir.ActivationFunctionType.Sign,
                                    bias=nth_sb[:, t:t + 1], scale=1.0)
                                nc.sync.dma_start(spk[t, b], s_t[b][:])
                        u_prev = u_cur
                        xcur = xnext
    nc.compile()
    return nc


# --------------------------------------------------------------------------- #
# host-side input prep
# --------------------------------------------------------------------------- #

def _prep_classic(mode, x, conv_w, conv_b, bn_gamma, bn_beta, bn_mean, bn_var,
                  alpha, lif_beta, theta, v0):
    a = float(np.asarray(alpha).reshape(-1)[0])
    bb = float(np.asarray(lif_beta).reshape(-1)[0])
    th = float(np.asarray(theta).reshape(-1)[0])
    vr = float(np.asarray(v0).reshape(-1)[0])
    assert vr == 0.0, "fast path assumes v0 == 0"
    assert a > 0.0, "fast path assumes alpha > 0"

    scale = (bn_gamma / np.sqrt(bn_var + BN_EPS)).astype(np.float32)
    wf = (conv_w * scale[:, None, None, None]).astype(np.float32)   # [CO,CI,3,3]
    biasf = ((conv_b - bn_mean) * scale + bn_beta).astype(np.float32)

    fac = np.array([a ** (-t) for t in range(T)], dtype=np.float64)
    th_ts = tuple(float(np.float32(th * fac[t])) for t in range(T))

    # bias per t (scaled); t=0 additionally gets a*vr (zero here)
    bias_np = np.empty((CO, T), np.float32)
    for t in range(T):
        bias_np[:, t] = ((biasf + bb) * fac[t]).astype(np.float32)
    bias_np[:, 0] += np.float32(a * vr)

    # padded input
    xpad = np.zeros((T, B, CI, HP, WP), np.float32)
    xpad[:, :, :, 1:H + 1, 1:W + 1] = np.asarray(x, dtype=np.float32)

    ins = {}
    if mode in ("f32r", "f32"):
        ins["xp"] = xpad
        wp = np.empty((128, T, 3, CO), np.float32)
        ws = np.empty((64, T, 3, CO), np.float32)
        for t in range(T):
            wt = (wf * np.float32(fac[t])).astype(np.float32)
            for dx in range(3):
                wp[0:64, t, dx, :] = wt[:, :, 0, dx].T
                wp[64:128, t, dx, :] = wt[:, :, 1, dx].T
                ws[:, t, dx, :] = wt[:, :, 2, dx].T
        ins["wp"], ins["ws"] = wp, ws
    else:
        xh = xpad.astype(BF16)
        xl = (xpad - xh.astype(np.float32)).astype(BF16)
        ins["xh"], ins["xl"] = xh, xl
        w3 = np.empty((128, T, 9, CO), BF16)
        wc = np.empty((128, T, 3, CO), BF16)
        wcs = np.empty((64, T, 3, CO), BF16)
        for t in range(T):
            wt = (wf * np.float32(fac[t])).astype(np.float32)
            wh = wt.astype(BF16)
            wl = (wt - wh.astype(np.float32)).astype(BF16)
            for dy in range(3):
                for dx in range(3):
                    w3[0:64, t, 3 * dy + dx, :] = wh[:, :, dy, dx].T
                    w3[64:128, t, 3 * dy + dx, :] = wh[:, :, dy, dx].T
            for dx in range(3):
                wc[0:64, t, dx, :] = wl[:, :, 0, dx].T
                wc[64:128, t, dx, :] = wl[:, :, 1, dx].T
                wcs[:, t, dx, :] = wl[:, :, 2, dx].T
        ins["w3"], ins["wc"], ins["wcs"] = w3, wc, wcs

    ins["bias_in"] = bias_np
    return ins, th_ts


def _prep_bp(x, conv_w, conv_b, bn_gamma, bn_beta, bn_mean, bn_var,
             alpha, lif_beta, theta, v0):
    a = float(np.asarray(alpha).reshape(-1)[0])
    bb = float(np.asarray(lif_beta).reshape(-1)[0])
    th = float(np.asarray(theta).reshape(-1)[0])
    vr = float(np.asarray(v0).reshape(-1)[0])
    assert vr == 0.0 and a > 0.0

    scale = (bn_gamma / np.sqrt(bn_var + BN_EPS)).astype(np.float32)
    wf = (conv_w * scale[:, None, None, None]).astype(np.float32)
    biasf = ((conv_b - bn_mean) * scale + bn_beta).astype(np.float32)

    fac = np.array([a ** (-t) for t in range(T)], dtype=np.float64)
    th_ts = tuple(float(np.float32(th * fac[t])) for t in range(T))

    bias_np = np.empty((CO, T), np.float32)
    for t in range(T):
        bias_np[:, t] = ((biasf + bb) * fac[t]).astype(np.float32)
    bias_np[:, 0] += np.float32(a * vr)

    mnp = np.float16 if BP_F16 else np.float32
    xpad = np.zeros((T, B, CI, HP, WP), mnp)
    xpad[:, :, :, 1:H + 1, 1:W + 1] = np.asarray(x, dtype=np.float32)

    wb = np.empty((128, T, 9, CO), mnp)
    for t in range(T):
        wt = (wf * np.float32(fac[t])).astype(np.float32)
        for dy in range(3):
            for dx in range(3):
                wb[0:64, t, 3 * dy + dx, :] = wt[:, :, dy, dx].T
                wb[64:128, t, 3 * dy + dx, :] = wt[:, :, dy, dx].T
    nth = np.empty((CO, T), np.float32)
    for t in range(T):
        nth[:, t] = -np.float32(th_ts[t])
    return {"xp": xpad, "wb": wb, "bias_in": bias_np, "nth_in": nth}, th_ts


def _in_maps(mode, ins):
    """Split full prepped inputs into 8 per-core maps (shard along B)."""
    maps = []
    for c in range(N_CORES):
        m = {}
        for k, v in ins.items():
            if k in ("xp", "xh", "xl"):
                m[k] = np.ascontiguousarray(v[:, c * BLOC:(c + 1) * BLOC])
            else:
                m[k] = v
        maps.append(m)
    return maps


# --------------------------------------------------------------------------- #
# public entry
# --------------------------------------------------------------------------- #

_CACHE = {}


def _get_module(mode, th_ts, repeat=1, hw_loop=0):
    key = (mode, th_ts, repeat, hw_loop, SBUF_COPY, SPK_BF16, S_ON_ACT)
    if key not in _CACHE:
        if mode == "bp":
            _CACHE[key] = build_module_bp(th_ts, repeat, hw_loop)
        else:
            _CACHE[key] = build_module(mode, th_ts, repeat, hw_loop)
    return _CACHE[key]


def _prep(mode, **inputs):
    if mode == "bp":
        return _prep_bp(**inputs)
    return _prep_classic(mode, **inputs)


def _run(mode, inputs, repeat=1):
    ins, th_ts = _prep(mode, **inputs)
    nc = _get_module(mode, th_ts, repeat)
    maps = _in_maps(mode, ins)
    res = run_bass_kernel_spmd(nc, maps, core_ids=list(range(N_CORES)))
    out = np.concatenate([r["spk"] for r in res.results], axis=1)
    if mode == "bp":
        out = (np.asarray(out).astype(np.float32) > 0.0)
    return out.reshape(T, B, CO, H, W).astype(np.float32)


def kernel(**inputs):
    inputs = {k: np.asarray(v) for k, v in inputs.items()}
    return _run(MODE, inputs)

